# revision 1
# baseline (speedup 1.0000x reference)
"""Trainium2 Bass kernel for nn_AttentionBlock (sliding-window attention block).

Distribution: tensor-parallel over the 8 KV head groups (one group per core).
Each core computes qkv^T for its group (640 rows x 2048 tokens), windowed
attention for its 8 q-heads, and a partial output projection; host sums the
8 bf16 partials and adds x + b_out.

Device layout is feature-major: features on partitions, tokens on the free
dim.  Host pre-transposes x (as bf16) and the weights.

Key structure:
- x shipped bf16, host-swizzled to [128, 23, T] so each 512-token chunk loads
  with a single DMA descriptor; rmsnorm denominator rows (s, sqrtH/s) are
  host-precomputed, with the qkv bias folded in via the u-row trick.
- GQA batching: all 8 q-heads share one k/v head, so scores and prob@v run as
  N=512 matmuls over 4-head groups (q layout [64, head, tok]).
- per token-block, both head-groups' score matmuls are emitted before either
  group's prob@v, so the exp (scalar engine) hides under the other group's
  score matmuls instead of stalling the PE.
- the next chunk's x load + qkv matmuls are emitted between this chunk's rope
  and attention, keeping the PE busy through the chunk boundary.
- sink logits enter the softmax denominator via a rank-1 matmul into the
  prob-sum row of the attention psum tile.
- out-projection runs after the token loop with each wo tile reused across
  all 4 token chunks back-to-back; partials drain as bf16 and the host sums
  the 8 partials in float64.
"""

import math

import numpy as np
import ml_dtypes

import concourse.bass as bass
import concourse.mybir as mybir
import concourse.tile as tile
from concourse import bacc, bass_utils

# ---- problem config (hardcoded from the reference) ----
HIDDEN = 2880
HEAD_DIM = 64
N_HEADS = 64
N_KV = 8
Q_MULT = N_HEADS // N_KV  # 8
SLIDING_WINDOW = 128
ROPE_BASE = 150000.0
ROPE_SCALE = 32.0
NTK_ALPHA = 1.0
NTK_BETA = 32.0
INIT_CTX = 4096
RMS_EPS = 1e-5
SM_SCALE = 1.0 / math.sqrt(HEAD_DIM)
Q_DIM = N_HEADS * HEAD_DIM  # 4096
KV_DIM = N_KV * HEAD_DIM  # 512
B, T = 1, 2048

N_CORES = 8
P = 128
TC = 512  # token chunk
NCH = T // TC  # 4 chunks
NT = TC // P  # 4 token blocks per chunk
KH = 23  # hidden tiles: 22x128 + 1x64 (+u row)
HID_SIZES = [128] * 22 + [64]
W_SIZES = [128] * 22 + [65]  # last tile carries the bias row
QKV_ROWS = Q_MULT * HEAD_DIM + 2 * HEAD_DIM  # 640
QKV_M = QKV_ROWS // P  # 5
CD = 23  # out-proj c tiles: 22x128 + 1x64
C_SIZES = [128] * 22 + [64]

F32 = mybir.dt.float32
F32R = mybir.dt.float32r
BF16 = mybir.dt.bfloat16
AF = mybir.ActivationFunctionType

_CACHE = {}


# ------------------------- host-side preparation -------------------------

def _rope_tables():
    d_half = HEAD_DIM // 2
    freq = ROPE_BASE ** (np.arange(0, HEAD_DIM, 2, dtype=np.float64) / HEAD_DIM)
    concentration = 0.1 * math.log(ROPE_SCALE) + 1.0
    low = d_half * math.log(INIT_CTX / (NTK_BETA * 2 * math.pi)) / math.log(ROPE_BASE)
    high = d_half * math.log(INIT_CTX / (NTK_ALPHA * 2 * math.pi)) / math.log(ROPE_BASE)
    interpolation = 1.0 / (ROPE_SCALE * freq)
    extrapolation = 1.0 / freq
    ramp = (np.arange(d_half, dtype=np.float64) - low) / (high - low)
    mask = 1.0 - np.clip(ramp, 0.0, 1.0)
    inv_freq = interpolation * (1.0 - mask) + extrapolation * mask
    pos = np.arange(T, dtype=np.float64)
    angles = pos[:, None] * inv_freq[None, :]  # [T, 32]
    cos = (np.cos(angles) * concentration).astype(np.float32)
    sin = (np.sin(angles) * concentration).astype(np.float32)
    return cos.T.copy(), sin.T.copy()  # [32, T]


def _perm64():
    # evens then odds within a 64-dim head
    return np.concatenate([np.arange(0, 64, 2), np.arange(1, 64, 2)])


def _host_prepare(x, norm_scale, w_qkv, b_qkv, sinks, w_out, b_out):
    x64 = x[0].astype(np.float64)
    s = np.sqrt((x64 * x64).sum(axis=1) + HIDDEN * RMS_EPS)  # [2048]
    rsh = (1.0 / s).reshape(1, -1).astype(np.float32)
    xf = np.zeros((KH * P, T), dtype=np.float32)
    xf[:HIDDEN] = x[0].T
    xf[HIDDEN] = s.astype(np.float32)  # u row = row 64 of tile 22
    xb = np.ascontiguousarray(
        xf.reshape(KH, P, T).transpose(1, 0, 2)).astype(ml_dtypes.bfloat16)

    w_eff = (w_qkv * norm_scale[None, :]).astype(np.float32)
    b_eff = b_qkv.astype(np.float32).copy()
    # fold softmax scale into q rows (rope is a rotation; scale commutes)
    w_eff[:Q_DIM] *= SM_SCALE
    b_eff[:Q_DIM] *= SM_SCALE
    # bias applied via u-row trick: psum += b_row x sqrt(sum_sq + H*eps)
    b_eff = b_eff / math.sqrt(HIDDEN)

    perm = _perm64()
    cosT, sinT = _rope_tables()
    sin_signed = np.concatenate([-sinT, sinT], axis=0)  # [64, T]

    # band masks for the two score blocks, repeated over 4 heads
    pidx = np.arange(P)[:, None]
    jidx = np.arange(P)[None, :]
    mask_a = np.tile((jidx >= pidx), (1, 4)).astype(ml_dtypes.bfloat16)  # [128, 512]
    mask_b = np.tile((jidx <= pidx), (1, 4)).astype(ml_dtypes.bfloat16)

    per_core = []
    for g in range(N_CORES):
        rows = []
        for h in range(Q_MULT):  # q heads of this group, rope-permuted
            base = (g * Q_MULT + h) * HEAD_DIM
            rows.append(base + perm)
        rows.append(Q_DIM + g * HEAD_DIM + perm)  # k head, rope-permuted
        rows.append(Q_DIM + KV_DIM + g * HEAD_DIM + np.arange(HEAD_DIM))  # v natural
        rows = np.concatenate(rows)
        wq_g = np.concatenate(
            [w_eff[rows].T, b_eff[rows][None, :]], axis=0
        ).astype(ml_dtypes.bfloat16)  # [2881, 640]
        wo_g = np.ascontiguousarray(
            w_out[:, g * KV_DIM:(g + 1) * KV_DIM].T
        ).astype(ml_dtypes.bfloat16)  # [512, 2880]
        sexp = np.exp2(sinks[g * Q_MULT:(g + 1) * Q_MULT]).astype(np.float32)
        srow = np.repeat(sexp, P).reshape(1, -1).astype(ml_dtypes.bfloat16)  # [1, 1024]
        per_core.append({
            "xb": xb,
            "rsh": rsh,
            "wq": wq_g,
            "wo": wo_g,
            "srow": srow,
            "cosT": cosT.astype(ml_dtypes.bfloat16),
            "sinS": sin_signed.astype(ml_dtypes.bfloat16),
            "mask_a": mask_a,
            "mask_b": mask_b,
            "sqrth": np.full((1, P), math.sqrt(HIDDEN), dtype=np.float32),
            "ones64": np.ones((1, HEAD_DIM), dtype=np.float32),
        })
    return per_core


# ------------------------- device program -------------------------

def build_program():
    nc = bacc.Bacc(None, target_bir_lowering=False)

    xb_d = nc.declare_dram_parameter("xb", [P, KH, T], BF16, isOutput=False)
    rsh_d = nc.declare_dram_parameter("rsh", [1, T], F32, isOutput=False)
    wq_d = nc.declare_dram_parameter("wq", [HIDDEN + 1, QKV_ROWS], BF16, isOutput=False)
    wo_d = nc.declare_dram_parameter("wo", [KV_DIM, HIDDEN], BF16, isOutput=False)
    srow_d = nc.declare_dram_parameter("srow", [1, 2 * TC], BF16, isOutput=False)
    cos_d = nc.declare_dram_parameter("cosT", [32, T], BF16, isOutput=False)
    sin_d = nc.declare_dram_parameter("sinS", [64, T], BF16, isOutput=False)
    maska_d = nc.declare_dram_parameter("mask_a", [P, TC], BF16, isOutput=False)
    maskb_d = nc.declare_dram_parameter("mask_b", [P, TC], BF16, isOutput=False)
    sqrth_d = nc.declare_dram_parameter("sqrth", [1, P], F32R, isOutput=False)
    ones64_d = nc.declare_dram_parameter("ones64", [1, HEAD_DIM], F32R, isOutput=False)
    out_d = nc.declare_dram_parameter("partial", [HIDDEN, T], BF16, isOutput=True)

    with tile.TileContext(nc) as tc:
        _body(tc, nc, xb_d, rsh_d, wq_d, wo_d, srow_d, cos_d, sin_d,
              maska_d, maskb_d, sqrth_d, ones64_d, out_d)
    nc.compile()
    return nc


def _body(tc, nc, xb_d, rsh_d, wq_d, wo_d, srow_d, cos_d, sin_d,
          maska_d, maskb_d, sqrth_d, ones64_d, out_d):
    import contextlib
    ctx = contextlib.ExitStack()
    with ctx:
        const = ctx.enter_context(tc.tile_pool(name="const", bufs=1))
        xbf = ctx.enter_context(tc.tile_pool(name="xbf", bufs=2))
        small = ctx.enter_context(tc.tile_pool(name="small", bufs=1))
        qkvp = ctx.enter_context(tc.tile_pool(name="qkvp", bufs=7))
        qsp = ctx.enter_context(tc.tile_pool(name="qsp", bufs=2))
        krp = ctx.enter_context(tc.tile_pool(name="krp", bufs=2))
        kprevp = ctx.enter_context(tc.tile_pool(name="kprevp", bufs=2))
        tmpp = ctx.enter_context(tc.tile_pool(name="tmpp", bufs=2))
        vlowp = ctx.enter_context(tc.tile_pool(name="vlowp", bufs=1))
        vnatp = ctx.enter_context(tc.tile_pool(name="vnatp", bufs=6))
        expp = ctx.enter_context(tc.tile_pool(name="expp", bufs=4))
        probp = ctx.enter_context(tc.tile_pool(name="probp", bufs=6))
        rsbp = ctx.enter_context(tc.tile_pool(name="rsbp", bufs=2))
        invbp = ctx.enter_context(tc.tile_pool(name="invbp", bufs=2))
        attnp = ctx.enter_context(tc.tile_pool(name="attnp", bufs=1))
        outsb = ctx.enter_context(tc.tile_pool(name="outsb", bufs=2))

        # ---- small constants first (so chunk-0 x loads aren't stuck behind
        # the big weight DMAs; wq issues after chunk-0 x, wo at end of ch 0) ----
        onescol = const.tile([P, 1], BF16)
        nc.vector.memset(onescol[:], 1.0)
        eps_bias = const.tile([1, 1], F32)
        nc.vector.memset(eps_bias[:], float(HIDDEN * RMS_EPS))
        e65 = const.tile([1, 65], BF16)
        nc.vector.memset(e65[:], 0.0)
        nc.vector.memset(e65[0:1, 64:65], 1.0)
        sqrt_h_row = const.tile([1, P], F32R)
        nc.sync.dma_start(out=sqrt_h_row[:], in_=sqrth_d[:])
        rsh_sb = const.tile([1, T], F32)
        nc.sync.dma_start(out=rsh_sb[:], in_=rsh_d[:])
        ones_p0 = const.tile([1, HEAD_DIM], F32R)
        nc.sync.dma_start(out=ones_p0[:], in_=ones64_d[:])
        srow_sb = const.tile([1, 2 * TC], BF16)
        nc.sync.dma_start(out=srow_sb[:], in_=srow_d[:])
        mask_a = const.tile([P, TC], BF16)
        nc.sync.dma_start(out=mask_a[:], in_=maska_d[:])
        mask_b = const.tile([P, TC], BF16)
        nc.sync.dma_start(out=mask_b[:], in_=maskb_d[:])
        cos_sb = const.tile([P, T], BF16)
        sin_sb = const.tile([P, T], BF16)

        wq_sb = const.tile([P, KH, QKV_ROWS], BF16)
        wo_sb = const.tile([P, 4, HIDDEN], BF16)

        loop_ps = contextlib.ExitStack()
        ps_mm = loop_ps.enter_context(tc.tile_pool(name="ps_mm", bufs=2, space="PSUM"))
        ps_sc = loop_ps.enter_context(tc.tile_pool(name="ps_sc", bufs=4, space="PSUM"))
        ps_av = loop_ps.enter_context(tc.tile_pool(name="ps_av", bufs=2, space="PSUM"))

        kprev = None   # [64, P] k of last block of previous chunk
        vnat_prev = None
        attn_all = [None] * (NCH * 4)

        def emit_AB(ch):
            t0 = ch * TC
            # ---- phase A: load x chunk (bf16, swizzled [p, k, t]) ----
            # chunk 0 splits into 4 DMAs so qkv can start before the full
            # 2.95MB lands; prefetched chunks use a single descriptor
            xall = xbf.tile([P, KH, TC], BF16, tag="xk", name=f"xall_{ch}")
            if ch == 0:
                for k0, k1 in ((0, 6), (6, 12), (12, 18), (18, KH)):
                    nc.sync.dma_start(out=xall[:, k0:k1, :],
                                      in_=xb_d[:, k0:k1, t0:t0 + TC])
            else:
                nc.sync.dma_start(out=xall[:, :, :], in_=xb_d[:, :, t0:t0 + TC])
            xb_tiles = [xall[:, k, :] for k in range(KH)]
            if ch == 0:
                for k in range(KH):
                    nc.scalar.dma_start(out=wq_sb[0:W_SIZES[k], k, :],
                                        in_=wq_d[k * P:k * P + W_SIZES[k], :])
                for b in range(4):
                    nc.scalar.dma_start(out=cos_sb[32 * b:32 * (b + 1), :], in_=cos_d[:])
                for b in range(2):
                    nc.scalar.dma_start(out=sin_sb[64 * b:64 * (b + 1), :], in_=sin_d[:])
            rr = small.tile([1, TC], F32R, tag="rr", name=f"rr_{ch}")
            nc.vector.tensor_copy(rr[:], rsh_sb[0:1, t0:t0 + TC])
            pinv = ps_sc.tile([P, TC], F32, tag="sc", name=f"pinv_{ch}")
            nc.tensor.matmul(pinv[:], sqrt_h_row[:], rr[:])
            invb = invbp.tile([P, TC], F32, tag="invb", name=f"invb_{ch}")
            nc.scalar.activation(invb[:], pinv[:], AF.Copy)

            # ---- phase B: qkv^T matmul (bias folded), normalize on drain ----
            qkv_tiles = []
            for m in range(QKV_M):
                pq = ps_mm.tile([P, TC], F32, tag="mm", name=f"pq_{ch}_{m}")
                for k in range(KH):
                    ws = W_SIZES[k]
                    nc.tensor.matmul(pq[:], wq_sb[0:ws, k, m * P:(m + 1) * P],
                                     xb_tiles[k][0:ws, :],
                                     start=(k == 0), stop=(k == KH - 1))
                qm = qkvp.tile([P, TC], BF16, tag="qkv", name=f"qm_{ch}_{m}")
                nc.vector.tensor_mul(qm[:], pq[:], invb[:])
                qkv_tiles.append(qm)
            return qkv_tiles

        ab_state = {0: emit_AB(0)}
        for ch in range(NCH):
            t0 = ch * TC
            qkv_tiles = ab_state.pop(ch)

            # ---- phase C: rope; q lands in [64, head, tok] layout ----
            qs = qsp.tile([64, Q_MULT, TC], BF16)
            kr = krp.tile([64, TC], BF16)
            for m in range(QKV_M):
                rows = P if m < 4 else HEAD_DIM
                src = qkv_tiles[m]
                sw = tmpp.tile([P, TC], BF16, tag="sw", bufs=1)
                for b in range(rows // 64):
                    nc.vector.tensor_copy(sw[64 * b:64 * b + 32, :],
                                          src[64 * b + 32:64 * b + 64, :])
                    nc.vector.tensor_copy(sw[64 * b + 32:64 * b + 64, :],
                                          src[64 * b:64 * b + 32, :])
                t1 = tmpp.tile([P, TC], BF16, tag="t1", bufs=1)
                nc.vector.tensor_mul(t1[0:rows, :], src[0:rows, :],
                                     cos_sb[0:rows, t0:t0 + TC])
                t2 = tmpp.tile([P, TC], BF16, tag="t2", bufs=1)
                nc.vector.tensor_mul(t2[0:rows, :], sw[0:rows, :],
                                     sin_sb[0:rows, t0:t0 + TC])
                if m < 4:
                    qr = tmpp.tile([P, TC], BF16, tag="qr", bufs=1)
                    nc.vector.tensor_add(qr[:], t1[:], t2[:])
                    nc.vector.tensor_copy(qs[:, 2 * m, :], qr[0:64, :])
                    nc.vector.tensor_copy(qs[:, 2 * m + 1, :], qr[64:P, :])
                else:
                    nc.vector.tensor_add(kr[:], t1[0:64, :], t2[0:64, :])

            # ---- phase D: v natural layout via DMA-xbar transpose ----
            v0 = vlowp.tile([HEAD_DIM, TC], BF16)
            nc.vector.tensor_copy(v0[:], qkv_tiles[4][HEAD_DIM:P, :])
            vnat_tiles = []
            for tau in range(NT):
                vn = vnatp.tile([P, 65], BF16, tag="vn")
                nc.vector.memset(vn[:, 64:65], 1.0)
                nc.sync.dma_start(out=vn[:, 0:HEAD_DIM], in_=v0[:, tau * P:(tau + 1) * P],
                                  transpose=True)
                vnat_tiles.append(vn)

            # next chunk's x load + qkv matmuls keep the PE busy through the
            # rope/transpose boundary of this chunk
            if ch + 1 < NCH:
                ab_state[ch + 1] = emit_AB(ch + 1)

            # ---- phase E: attention, 4-head groups ----
            attn_tiles = [attnp.tile([P, TC], BF16, name=f"attn_{ch}_{kk}")
                          for kk in range(4)]
            for kk in range(4):
                attn_all[ch * 4 + kk] = attn_tiles[kk]
            for tau in range(NT):
                tg = ch * NT + tau
                kprev_blk = (kr[:, (tau - 1) * P:tau * P] if tau > 0
                             else (kprev[:, :] if kprev is not None else None))
                vprev_blk = vnat_tiles[tau - 1] if tau > 0 else vnat_prev
                prs = []
                for hg in range(2):
                    qblk = qs[:, hg * 4:(hg + 1) * 4, tau * P:(tau + 1) * P]
                    psc_a = ps_sc.tile([P, TC], F32, tag="sc", name=f"pa_{hg}")
                    nc.tensor.matmul(psc_a[:], kr[:, tau * P:(tau + 1) * P], qblk,
                                     start=True, stop=True)
                    et_a = expp.tile([P, TC], BF16, tag="et", name=f"ea_{hg}")
                    nc.scalar.activation(et_a[:], psc_a[:], AF.Exp)
                    pr_a = probp.tile([P, TC], BF16, tag="pr", name=f"fa_{hg}")
                    nc.vector.tensor_mul(pr_a[:], et_a[:], mask_a[:])
                    pr_b = None
                    if tg > 0:
                        psc_b = ps_sc.tile([P, TC], F32, tag="sc", name=f"pb_{hg}")
                        nc.tensor.matmul(psc_b[:], kprev_blk, qblk,
                                         start=True, stop=True)
                        et_b = expp.tile([P, TC], BF16, tag="et", name=f"eb_{hg}")
                        nc.scalar.activation(et_b[:], psc_b[:], AF.Exp)
                        pr_b = probp.tile([P, TC], BF16, tag="pr", name=f"fb_{hg}")
                        nc.vector.tensor_mul(pr_b[:], et_b[:], mask_b[:])
                    prs.append((pr_a, pr_b))

                for hg in range(2):
                    pr_a, pr_b = prs[hg]
                    pav = ps_av.tile([65, TC], F32, tag="av", name=f"pv_{hg}")
                    nc.tensor.matmul(pav[:], vnat_tiles[tau][:, 0:65], pr_a[:],
                                     start=True, stop=False)
                    nc.tensor.matmul(pav[:], e65[:],
                                     srow_sb[0:1, hg * TC:(hg + 1) * TC],
                                     start=False, stop=(tg == 0))
                    if tg > 0:
                        nc.tensor.matmul(pav[:], vprev_blk[:, 0:65], pr_b[:],
                                         start=False, stop=True)

                    dsum = small.tile([1, TC], F32, tag="dsum", name=f"dm_{hg}")
                    nc.vector.tensor_copy(dsum[:], pav[64:65, :])
                    rrh = small.tile([1, TC], F32, tag="rrh", name=f"rh_{hg}")
                    nc.vector.reciprocal_approx_fast(rrh[:], dsum[:])
                    rrhr = small.tile([1, TC], F32R, tag="rrhr", name=f"rr_{hg}")
                    nc.vector.tensor_copy(rrhr[:], rrh[:])
                    prb = ps_sc.tile([64, TC], F32, tag="sc", name=f"pr_{hg}")
                    nc.tensor.matmul(prb[:], ones_p0[:], rrhr[:])
                    rsb = rsbp.tile([HEAD_DIM, TC], F32, tag="rsb", name=f"rs_{hg}")
                    nc.scalar.activation(rsb[:], prb[:], AF.Copy)
                    stage = probp.tile([HEAD_DIM, TC], BF16, tag="stage", bufs=2,
                                       name=f"sg_{hg}")
                    nc.vector.tensor_mul(stage[:], pav[0:HEAD_DIM, :], rsb[:])
                    for j in range(4):
                        h = hg * 4 + j
                        kk = h // 2
                        rlo = 64 * (h % 2)
                        eng = nc.vector if j % 2 == 0 else nc.gpsimd
                        eng.tensor_copy(
                            attn_tiles[kk][rlo:rlo + HEAD_DIM, tau * P:(tau + 1) * P],
                            stage[:, j * P:(j + 1) * P])

            kprev_t = kprevp.tile([HEAD_DIM, P], BF16)
            nc.vector.tensor_copy(kprev_t[:], kr[:, TC - P:TC])
            kprev = kprev_t
            vnat_prev = vnat_tiles[NT - 1]

            if ch == 0:
                for k in range(4):
                    nc.sync.dma_start(out=wo_sb[:, k, :], in_=wo_d[k * P:(k + 1) * P, :])

        # ---- phase F: out-projection over all chunks, wo tiles reused ----
        loop_ps.close()
        ps_op = ctx.enter_context(tc.tile_pool(name="ps_op", bufs=8, space="PSUM"))
        for c in range(CD):
            cs = C_SIZES[c]
            po = [ps_op.tile([P, TC], F32, tag="op", name=f"po_{c}_{i}")
                  for i in range(NCH)]
            for kk in range(4):
                for ch in range(NCH):
                    nc.tensor.matmul(po[ch][0:cs, :], wo_sb[:, kk, c * P:c * P + cs],
                                     attn_all[ch * 4 + kk][:],
                                     start=(kk == 0), stop=(kk == 3))
            ot = outsb.tile([P, NCH, TC], BF16, tag="ot")
            for ch in range(NCH):
                if ch % 2 == 0:
                    nc.scalar.activation(ot[0:cs, ch, :], po[ch][0:cs, :], AF.Copy)
                else:
                    nc.vector.tensor_copy(ot[0:cs, ch, :], po[ch][0:cs, :])
            nc.sync.dma_start(out=out_d[c * P:c * P + cs, :], in_=ot[0:cs, :, :])




# ------------------------- entry point -------------------------

def _get_program():
    if "nc" not in _CACHE:
        _CACHE["nc"] = build_program()
    return _CACHE["nc"]


def run_cores(inputs, trace=False):
    per_core = _host_prepare(**inputs)
    nc = _get_program()
    res = bass_utils.run_bass_kernel_spmd(
        nc, per_core, core_ids=list(range(N_CORES)), trace=trace,
    )
    return res


def kernel(**inputs):
    res = run_cores(inputs)
    acc = np.zeros((HIDDEN, T), dtype=np.float64)
    for r in res.results:
        acc += np.asarray(r["partial"], dtype=np.float64)
    out = acc.T + inputs["x"][0].astype(np.float64) + inputs["b_out"][None, :].astype(np.float64)
    return out[None].astype(np.float32)

